# revision 3
# baseline (speedup 1.0000x reference)
"""Multi-head attention block (b=8, n=1024, d=1024, heads=16) on 8 trn2
NeuronCores, data-parallel over batch (one batch element per core).

Per-core dataflow (all f32, all matmuls on PE):
  B:  qkT[c, t]  = sum_d WqkvT[d, c] * xT[d, t]      (q,k channels 0..2047)
  C:  V[t, c]    = sum_d xT[d, t]    * WqkvT[d, 2048+c]
  D:  per head h (d_h = 64):
        S^T[j, i] = sum_d kT[d, j] qT[d, i]           (K=64 matmul)
        E = exp(S^T * scale)                          (ACT, no max-subtract:
                                                       |scores*scale| < ~3)
        [O^T_u; rowsum] = [V_h | 1]^T E               (ones column appended to
                                                       V gives rowsum for free)
        O^T = O^T_u * (1/rowsum broadcast)            (broadcast via K=1 PE
                                                       outer product ones x r)
  E:  yT[o, t] = sum_D WprojT[D, o] O^T[D, t] + bias[o]

Layout trick: softmax normalization needs a per-column scale on O^T_u; the
reciprocal row sits on PSUM partition 64, is broadcast to [64, 512] with a
K=1 matmul, then one DVE multiply normalizes. Odd heads land on SBUF
partitions 64..127 of the O^T tile via a SBUF->SBUF DMA (DVE lanes are
partition-local and cannot shift partitions).

Host does only data movement: transposes / tiling rearranges of x and the
weights, and the inverse transpose of the output.
"""

import json

import numpy as np

D = 1024
NT = 1024
H = 16
HD = 64
P = 128
DC = D // P  # 8 contraction chunks
SCALE = HD ** -0.5
N_CORES = 8

_CACHE = {}


# --------------------------------------------------------------------------
# Workaround for the walrus build in this container: each TPB instruction
# encodes at most ONE sync wait (NEURON_ISA_TPB_EVENTS has a single wait
# slot) and this walrus version errors out instead of splitting. Tile
# attaches several waits per instruction. Hoist all but the last wait onto
# preceding single-wait EventSemaphore no-ops on the same (in-order) engine.
# --------------------------------------------------------------------------
def _split_sync_waits_json(bir_bytes: bytes) -> bytes:
    j = json.loads(bir_bytes)
    changed = False
    ctr = 0
    dma_ops = {"TensorLoad", "TensorSave", "TensorCopy", "TensorReduce"}
    for fn in j.get("functions", []):
        for blk in fn.get("blocks", []):
            out = []
            for inst in blk.get("instructions", []):
                si = inst.get("sync_info")
                if si:
                    waits = si.get("on_wait") or []
                    if len(waits) > 1:
                        for w in waits[:-1]:
                            ctr += 1
                            out.append(
                                {
                                    "debug": inst.get("debug", 0),
                                    "engine": inst.get("engine"),
                                    "ins": [],
                                    "outs": [],
                                    "name": f"splitw-{ctr}-{inst['name']}",
                                    "opcode": "EventSemaphore",
                                    "sync_info": {"on_update": [], "on_wait": [w]},
                                }
                            )
                        si["on_wait"] = [waits[-1]]
                        changed = True
                    ups = si.get("on_update") or []
                    if len(ups) > 1 and inst.get("opcode") not in dma_ops:
                        extra = ups[:-1]
                        si["on_update"] = [ups[-1]]
                        out.append(inst)
                        for u in extra:
                            ctr += 1
                            out.append(
                                {
                                    "debug": inst.get("debug", 0),
                                    "engine": inst.get("engine"),
                                    "ins": [],
                                    "outs": [],
                                    "name": f"splitu-{ctr}-{inst['name']}",
                                    "opcode": "EventSemaphore",
                                    "sync_info": {"on_update": [u], "on_wait": []},
                                }
                            )
                        changed = True
                        continue
                out.append(inst)
            blk["instructions"] = out
    if not changed:
        return bir_bytes
    return json.dumps(j).encode()


def _install_bir_fix():
    import concourse.bass as bass

    if getattr(bass.Bass, "_split_waits_patched", False):
        return
    orig = bass.Bass.to_json_bytes

    def patched(self, *a, **kw):
        return _split_sync_waits_json(orig(self, *a, **kw))

    bass.Bass.to_json_bytes = patched
    bass.Bass._split_waits_patched = True


def _build_module():
    from contextlib import ExitStack

    import concourse.bass as bass
    import concourse.tile as tile
    from concourse import mybir

    _install_bir_fix()
    f32 = mybir.dt.float32
    nc = bass.Bass()

    xT = nc.declare_dram_parameter("xT", [D, NT], f32, isOutput=False)
    # wqk[p, ct, a, c] = W_qkv.T[a*128+p, ct*128+c]  (q,k channels, ct<16)
    wqk = nc.declare_dram_parameter("wqk", [P, 16, DC, P], f32, isOutput=False)
    # wv[p, a, cv] = W_qkv.T[a*128+p, 2048+cv]
    wvp = nc.declare_dram_parameter("wv", [P, DC, D], f32, isOutput=False)
    # wpr[p, ot, a, c] = W_proj.T[a*128+p, ot*128+c]
    wpr = nc.declare_dram_parameter("wpr", [P, DC, DC, P], f32, isOutput=False)
    # biasT[p, t] = b_proj[t*128+p]
    biasT = nc.declare_dram_parameter("biasT", [P, DC], f32, isOutput=False)
    yT = nc.declare_dram_parameter("yT", [D, NT], f32, isOutput=True)

    with tile.TileContext(nc) as tc, ExitStack() as outer:
        qk_pool = outer.enter_context(tc.tile_pool(name="qk", bufs=1))
        v_pool = outer.enter_context(tc.tile_pool(name="vsb", bufs=1))
        misc = outer.enter_context(tc.tile_pool(name="misc", bufs=1))

        qk = qk_pool.tile([P, 16, NT], f32)       # q,k channel tiles
        v_sb = v_pool.tile([P, DC, H, HD + 1], f32)  # V + ones column per head
        ones_t = misc.tile([P, HD], f32)
        bias_t = misc.tile([P, DC], f32)
        nc.vector.memset(ones_t[:], 1.0)
        nc.gpsimd.dma_start(bias_t[:], biasT[:])
        nc.vector.memset(v_sb[:, :, :, HD], 1.0)

        # ---------------- stage B/C: qkv projection ----------------
        with (
            tc.tile_pool(name="xt", bufs=1) as xt_pool,
            tc.tile_pool(name="wvt", bufs=1) as wv_pool,
            tc.tile_pool(name="wt", bufs=3) as wt_pool,
            tc.tile_pool(name="psbc", bufs=4, space="PSUM") as psbc,
        ):
            xt = xt_pool.tile([P, DC, NT], f32)
            for a in range(DC):
                nc.gpsimd.dma_start(xt[:, a, :], xT[a * P : (a + 1) * P, :])
            wv = wv_pool.tile([P, DC, D], f32)
            nc.gpsimd.dma_start(wv[:], wvp[:])

            # V[t, c] (token partitions), channels in two 512 halves
            for vt in range(DC):
                for ch in range(2):
                    ps = psbc.tile([P, 512], f32, tag="psv")
                    for a in range(DC):
                        nc.tensor.matmul(
                            ps[:],
                            xt[:, a, vt * P : (vt + 1) * P],
                            wv[:, a, ch * 512 : (ch + 1) * 512],
                            start=(a == 0),
                            stop=(a == DC - 1),
                        )
                    for hh in range(8):
                        h = ch * 8 + hh
                        nc.vector.tensor_copy(
                            v_sb[:, vt, h, 0:HD], ps[:, hh * HD : (hh + 1) * HD]
                        )

            # qkT[c, t]; emit q/k tile pairs so attention heads unblock early
            ct_order = [t for pair in zip(range(8), range(8, 16)) for t in pair]
            for ct in ct_order:
                wt = wt_pool.tile([P, DC, P], f32)
                nc.gpsimd.dma_start(wt[:], wqk[:, ct, :, :])
                for nh in range(2):
                    ps = psbc.tile([P, 512], f32, tag="psqk")
                    for a in range(DC):
                        nc.tensor.matmul(
                            ps[:],
                            wt[:, a, :],
                            xt[:, a, nh * 512 : (nh + 1) * 512],
                            start=(a == 0),
                            stop=(a == DC - 1),
                        )
                    nc.vector.tensor_copy(qk[:, ct, nh * 512 : (nh + 1) * 512], ps[:])

        # ---------------- stages D+E ----------------
        with tc.tile_pool(name="otp", bufs=1) as ot_pool:
            ot = ot_pool.tile([P, DC, NT], f32)  # O^T, channel-major

            with (
                tc.tile_pool(name="es", bufs=8) as es_pool,
                tc.tile_pool(name="tmp", bufs=2) as tmp_pool,
                tc.tile_pool(name="rsp", bufs=2) as rs_pool,
                tc.tile_pool(name="rbp", bufs=2) as rb_pool,
                tc.tile_pool(name="psS", bufs=2, space="PSUM") as psS,
                tc.tile_pool(name="psO", bufs=2, space="PSUM") as psO,
                tc.tile_pool(name="psB", bufs=2, space="PSUM") as psB,
            ):
                for h in range(H):
                    qo = (h % 2) * HD
                    qs = qk[qo : qo + HD, h // 2, :]       # [64, 1024]
                    ks = qk[qo : qo + HD, 8 + h // 2, :]   # [64, 1024]

                    es_list = []
                    for j in range(DC):
                        sps = psS.tile([P, NT], f32)
                        for ih in range(2):
                            nc.tensor.matmul(
                                sps[:, ih * 512 : (ih + 1) * 512],
                                ks[:, j * P : (j + 1) * P],
                                qs[:, ih * 512 : (ih + 1) * 512],
                                start=True,
                                stop=True,
                            )
                        es = es_pool.tile([P, NT], f32)
                        nc.scalar.activation(
                            es[:], sps[:], mybir.ActivationFunctionType.Exp,
                            scale=SCALE,
                        )
                        es_list.append(es)

                    odd = h % 2 == 1
                    if odd:
                        tmp = tmp_pool.tile([HD, NT], f32)
                    else:
                        tmp = None
                    for ih in range(2):
                        ops = psO.tile([HD + 1, 512], f32)
                        for j in range(DC):
                            nc.tensor.matmul(
                                ops[:],
                                v_sb[:, j, h, :],
                                es_list[j][:, ih * 512 : (ih + 1) * 512],
                                start=(j == 0),
                                stop=(j == DC - 1),
                            )
                        rs = rs_pool.tile([P, 512], f32)
                        nc.vector.reciprocal(rs[HD : HD + 1, :], ops[HD : HD + 1, :])
                        bps = psB.tile([HD, 512], f32)
                        nc.tensor.matmul(
                            bps[:], ones_t[HD : HD + 1, :], rs[HD : HD + 1, :],
                            start=True, stop=True,
                        )
                        rb = rb_pool.tile([HD, 512], f32)
                        nc.vector.tensor_copy(rb[:], bps[:])
                        dst = (
                            tmp[:, ih * 512 : (ih + 1) * 512]
                            if odd
                            else ot[0:HD, h // 2, ih * 512 : (ih + 1) * 512]
                        )
                        nc.vector.tensor_mul(dst, ops[0:HD, :], rb[:])
                    if odd:
                        # DVE lanes cannot shift partitions; DMA moves the
                        # odd head's rows to partitions 64..127
                        nc.gpsimd.dma_start(ot[HD:P, h // 2, :], tmp[:])

            # -------- output projection + bias --------
            with (
                tc.tile_pool(name="wp", bufs=3) as wp_pool,
                tc.tile_pool(name="outp", bufs=3) as out_pool,
                tc.tile_pool(name="psE", bufs=4, space="PSUM") as psE,
            ):
                for oi in range(DC):
                    wpt = wp_pool.tile([P, DC, P], f32)
                    nc.gpsimd.dma_start(wpt[:], wpr[:, oi, :, :])
                    osb = out_pool.tile([P, NT], f32)
                    for nh in range(2):
                        ps = psE.tile([P, 512], f32)
                        for a in range(DC):
                            nc.tensor.matmul(
                                ps[:],
                                wpt[:, a, :],
                                ot[:, a, nh * 512 : (nh + 1) * 512],
                                start=(a == 0),
                                stop=(a == DC - 1),
                            )
                        nc.vector.tensor_scalar_add(
                            osb[:, nh * 512 : (nh + 1) * 512],
                            ps[:],
                            bias_t[:, oi : oi + 1],
                        )
                    nc.gpsimd.dma_start(yT[oi * P : (oi + 1) * P, :], osb[:])

    return nc


def _get_nc():
    if "nc" not in _CACHE:
        _CACHE["nc"] = _build_module()
    return _CACHE["nc"]


def _host_inputs(x, W_qkv, W_proj, b_proj):
    x = np.asarray(x, dtype=np.float32)
    W_qkv = np.asarray(W_qkv, dtype=np.float32)
    W_proj = np.asarray(W_proj, dtype=np.float32)
    b_proj = np.asarray(b_proj, dtype=np.float32)

    wqkvT = W_qkv.T  # [1024, 3072]
    # wqk[p, ct, a, c] = wqkvT[a*128+p, ct*128+c] for q,k channels
    wqk = np.ascontiguousarray(
        wqkvT[:, : 2 * D].reshape(DC, P, 16, P).transpose(1, 2, 0, 3)
    )
    # wv[p, a, cv] = wqkvT[a*128+p, 2048+cv]
    wv = np.ascontiguousarray(wqkvT[:, 2 * D :].reshape(DC, P, D).transpose(1, 0, 2))
    # wpr[p, ot, a, c] = W_proj.T[a*128+p, ot*128+c]
    wpr = np.ascontiguousarray(
        W_proj.T.reshape(DC, P, DC, P).transpose(1, 2, 0, 3)
    )
    biasT = np.ascontiguousarray(b_proj.reshape(DC, P).T)

    in_maps = []
    for i in range(N_CORES):
        in_maps.append(
            {
                "xT": np.ascontiguousarray(x[i].T),
                "wqk": wqk,
                "wv": wv,
                "wpr": wpr,
                "biasT": biasT,
            }
        )
    return in_maps


def _run(in_maps, trace=False):
    from concourse.bass_utils import run_bass_kernel_spmd

    nc = _get_nc()
    return run_bass_kernel_spmd(nc, in_maps, list(range(N_CORES)), trace=trace)


def kernel(x, W_qkv, W_proj, b_proj):
    in_maps = _host_inputs(x, W_qkv, W_proj, b_proj)
    res = _run(in_maps)
    out = np.stack([res.results[i]["yT"].T for i in range(N_CORES)], axis=0)
    return np.ascontiguousarray(out, dtype=np.float32)


# revision 7
# speedup vs baseline: 2.7070x; 2.7070x over previous
"""Multi-head attention block (b=8, n=1024, d=1024, heads=16) on 8 trn2
NeuronCores, data-parallel over batch (one batch element per core).

Per-core dataflow (all f32, all matmuls on PE):
  B:  qkT[c, t]  = sum_d WqkvT[d, c] * xT[d, t]      (q,k channels 0..2047)
  C:  V[t, c]    = sum_d xT[d, t]    * WqkvT[d, 2048+c]
  D:  per head h (d_h = 64):
        S^T[j, i] = sum_d kT[d, j] qT[d, i]           (K=64 matmul)
        E = exp(S^T * scale)                          (ACT, no max-subtract:
                                                       |scores*scale| < ~3)
        [O^T_u; rowsum] = [V_h | 1]^T E               (ones column appended to
                                                       V gives rowsum for free)
        O^T = O^T_u * (1/rowsum broadcast)            (broadcast via K=1 PE
                                                       outer product ones x r)
  E:  yT[o, t] = sum_D WprojT[D, o] O^T[D, t] + bias[o]

Layout trick: softmax normalization needs a per-column scale on O^T_u; the
reciprocal row sits on PSUM partition 64, is broadcast to [64, 512] with a
K=1 matmul, then one DVE multiply normalizes. Odd heads land on SBUF
partitions 64..127 of the O^T tile via a SBUF->SBUF DMA (DVE lanes are
partition-local and cannot shift partitions).

Host does only data movement: transposes / tiling rearranges of x and the
weights, and the inverse transpose of the output.
"""

import json

import numpy as np

D = 1024
NT = 1024
H = 16
HD = 64
P = 128
DC = D // P  # 8 contraction chunks
SCALE = HD ** -0.5
N_CORES = 8

_CACHE = {}


# --------------------------------------------------------------------------
# Workaround for the walrus build in this container: each TPB instruction
# encodes at most ONE sync wait (NEURON_ISA_TPB_EVENTS has a single wait
# slot) and this walrus version errors out instead of splitting. Tile
# attaches several waits per instruction. Hoist all but the last wait onto
# preceding single-wait EventSemaphore no-ops on the same (in-order) engine.
# --------------------------------------------------------------------------
def _split_sync_waits_json(bir_bytes: bytes) -> bytes:
    j = json.loads(bir_bytes)
    changed = False
    ctr = 0
    dma_ops = {"TensorLoad", "TensorSave", "TensorCopy", "TensorReduce"}
    for fn in j.get("functions", []):
        for blk in fn.get("blocks", []):
            out = []
            for inst in blk.get("instructions", []):
                si = inst.get("sync_info")
                if si:
                    waits = si.get("on_wait") or []
                    if len(waits) > 1:
                        for w in waits[:-1]:
                            ctr += 1
                            out.append(
                                {
                                    "debug": inst.get("debug", 0),
                                    "engine": inst.get("engine"),
                                    "ins": [],
                                    "outs": [],
                                    "name": f"splitw-{ctr}-{inst['name']}",
                                    "opcode": "EventSemaphore",
                                    "sync_info": {"on_update": [], "on_wait": [w]},
                                }
                            )
                        si["on_wait"] = [waits[-1]]
                        changed = True
                    ups = si.get("on_update") or []
                    if len(ups) > 1 and inst.get("opcode") not in dma_ops:
                        extra = ups[:-1]
                        si["on_update"] = [ups[-1]]
                        out.append(inst)
                        for u in extra:
                            ctr += 1
                            out.append(
                                {
                                    "debug": inst.get("debug", 0),
                                    "engine": inst.get("engine"),
                                    "ins": [],
                                    "outs": [],
                                    "name": f"splitu-{ctr}-{inst['name']}",
                                    "opcode": "EventSemaphore",
                                    "sync_info": {"on_update": [u], "on_wait": []},
                                }
                            )
                        changed = True
                        continue
                out.append(inst)
            blk["instructions"] = out
    if not changed:
        return bir_bytes
    return json.dumps(j).encode()


def _install_bir_fix():
    import concourse.bass as bass

    if getattr(bass.Bass, "_split_waits_patched", False):
        return
    orig = bass.Bass.to_json_bytes

    def patched(self, *a, **kw):
        return _split_sync_waits_json(orig(self, *a, **kw))

    bass.Bass.to_json_bytes = patched
    bass.Bass._split_waits_patched = True


def _build_module():
    from contextlib import ExitStack

    import concourse.bass as bass
    import concourse.tile as tile
    from concourse import mybir

    _install_bir_fix()
    f32 = mybir.dt.float32
    # fp32r: 4-byte fp32 operands streamed at bf16 rate (1 col/cycle for
    # N>=256) with ~1.4e-4 matmul rel error vs 4 cycles/col for true fp32.
    f32r = mybir.dt.float32r
    nc = bass.Bass()

    xT = nc.declare_dram_parameter("xT", [D, NT], f32, isOutput=False)
    # wqk[p, ct, a, c] = W_qkv.T[a*128+p, ct*128+c]  (q,k channels, ct<16)
    wqk = nc.declare_dram_parameter("wqk", [P, 16, DC, P], f32, isOutput=False)
    # wv[p, a, cv] = W_qkv.T[a*128+p, 2048+cv]
    wvp = nc.declare_dram_parameter("wv", [P, DC, D], f32, isOutput=False)
    # wpr[p, ot, a, c] = W_proj.T[a*128+p, ot*128+c]
    wpr = nc.declare_dram_parameter("wpr", [P, DC, DC, P], f32, isOutput=False)
    # biasT[p, t] = b_proj[t*128+p]
    biasT = nc.declare_dram_parameter("biasT", [P, DC], f32, isOutput=False)
    yT = nc.declare_dram_parameter("yT", [D, NT], f32, isOutput=True)

    with tile.TileContext(nc) as tc, ExitStack() as outer:
        qk_pool = outer.enter_context(tc.tile_pool(name="qk", bufs=1))
        v_pool = outer.enter_context(tc.tile_pool(name="vsb", bufs=1))
        misc = outer.enter_context(tc.tile_pool(name="misc", bufs=1))

        qk = qk_pool.tile([P, 16, NT], f32r)       # q,k channel tiles
        v_sb = v_pool.tile([P, DC, H, HD + 1], f32r)  # V + ones column per head
        ones_f = misc.tile([P, HD], f32)
        ones_t = misc.tile([P, HD], f32r)
        bias_t = misc.tile([P, DC], f32)
        nc.vector.memset(ones_f[:], 1.0)
        nc.vector.tensor_copy(ones_t[:], ones_f[:])
        nc.gpsimd.dma_start(bias_t[:], biasT[:])
        for vt in range(DC):
            nc.vector.tensor_copy(v_sb[:, vt, :, HD], ones_f[:, 0:H])

        # ---------------- stage B/C: qkv projection ----------------
        with (
            tc.tile_pool(name="xt", bufs=1) as xt_pool,
            tc.tile_pool(name="wvt", bufs=1) as wv_pool,
            tc.tile_pool(name="wt", bufs=3) as wt_pool,
            tc.tile_pool(name="psbc", bufs=4, space="PSUM") as psbc,
        ):
            xt = xt_pool.tile([P, DC, NT], f32r)
            for a in range(DC):
                nc.gpsimd.dma_start(xt[:, a, :], xT[a * P : (a + 1) * P, :])
            wv = wv_pool.tile([P, DC, D], f32r)
            nc.gpsimd.dma_start(wv[:], wvp[:])

            # V[t, c] (token partitions), channels in two 512 halves
            for vt in range(DC):
                for ch in range(2):
                    ps = psbc.tile([P, 512], f32, tag="psv")
                    for a in range(DC):
                        nc.tensor.matmul(
                            ps[:],
                            xt[:, a, vt * P : (vt + 1) * P],
                            wv[:, a, ch * 512 : (ch + 1) * 512],
                            start=(a == 0),
                            stop=(a == DC - 1),
                        )
                    for hh in range(8):
                        h = ch * 8 + hh
                        nc.vector.tensor_copy(
                            v_sb[:, vt, h, 0:HD], ps[:, hh * HD : (hh + 1) * HD]
                        )

            # qkT[c, t]; emit q/k tile pairs so attention heads unblock early
            ct_order = [t for pair in zip(range(8), range(8, 16)) for t in pair]
            for ct in ct_order:
                wt = wt_pool.tile([P, DC, P], f32r)
                nc.gpsimd.dma_start(wt[:], wqk[:, ct, :, :])
                for nh in range(2):
                    ps = psbc.tile([P, 512], f32, tag="psqk")
                    for a in range(DC):
                        nc.tensor.matmul(
                            ps[:],
                            wt[:, a, :],
                            xt[:, a, nh * 512 : (nh + 1) * 512],
                            start=(a == 0),
                            stop=(a == DC - 1),
                        )
                    nc.vector.tensor_copy(qk[:, ct, nh * 512 : (nh + 1) * 512], ps[:])

        # ---------------- stages D+E ----------------
        with tc.tile_pool(name="otp", bufs=1) as ot_pool:
            ot = ot_pool.tile([P, DC, NT], f32r)  # O^T, channel-major

            with (
                tc.tile_pool(name="es", bufs=8) as es_pool,
                tc.tile_pool(name="tmp", bufs=2) as tmp_pool,
                tc.tile_pool(name="rsp", bufs=2) as rs_pool,
                tc.tile_pool(name="rbp", bufs=2) as rb_pool,
                tc.tile_pool(name="psS", bufs=2, space="PSUM") as psS,
                tc.tile_pool(name="psO", bufs=2, space="PSUM") as psO,
                tc.tile_pool(name="psB", bufs=2, space="PSUM") as psB,
            ):
                for h in range(H):
                    qo = (h % 2) * HD
                    qs = qk[qo : qo + HD, h // 2, :]       # [64, 1024]
                    ks = qk[qo : qo + HD, 8 + h // 2, :]   # [64, 1024]

                    es_list = []
                    for j in range(DC):
                        sps = psS.tile([P, NT], f32)
                        for ih in range(2):
                            nc.tensor.matmul(
                                sps[:, ih * 512 : (ih + 1) * 512],
                                ks[:, j * P : (j + 1) * P],
                                qs[:, ih * 512 : (ih + 1) * 512],
                                start=True,
                                stop=True,
                            )
                        es = es_pool.tile([P, NT], f32r)
                        nc.scalar.activation(
                            es[:], sps[:], mybir.ActivationFunctionType.Exp,
                            scale=SCALE,
                        )
                        es_list.append(es)

                    odd = h % 2 == 1
                    if odd:
                        tmp = tmp_pool.tile([HD, NT], f32r)
                    else:
                        tmp = None
                    for ih in range(2):
                        ops = psO.tile([HD + 1, 512], f32)
                        for j in range(DC):
                            nc.tensor.matmul(
                                ops[:],
                                v_sb[:, j, h, :],
                                es_list[j][:, ih * 512 : (ih + 1) * 512],
                                start=(j == 0),
                                stop=(j == DC - 1),
                            )
                        # 1/rowsum = exp(-ln(rowsum)) -- two ACT table ops;
                        # DVE reciprocal is ~8 cycles/elem and far slower
                        rs = rs_pool.tile([P, 512], f32)
                        nc.scalar.activation(
                            rs[HD : HD + 1, :], ops[HD : HD + 1, :],
                            mybir.ActivationFunctionType.Ln,
                        )
                        rsr = rs_pool.tile([P, 512], f32r)
                        nc.scalar.activation(
                            rsr[HD : HD + 1, :], rs[HD : HD + 1, :],
                            mybir.ActivationFunctionType.Exp, scale=-1.0,
                        )
                        bps = psB.tile([HD, 512], f32)
                        nc.tensor.matmul(
                            bps[:], ones_t[HD : HD + 1, :], rsr[HD : HD + 1, :],
                            start=True, stop=True,
                        )
                        rb = rb_pool.tile([HD, 512], f32)
                        nc.vector.tensor_copy(rb[:], bps[:])
                        dst = (
                            tmp[:, ih * 512 : (ih + 1) * 512]
                            if odd
                            else ot[0:HD, h // 2, ih * 512 : (ih + 1) * 512]
                        )
                        nc.vector.tensor_mul(dst, ops[0:HD, :], rb[:])
                    if odd:
                        # DVE lanes cannot shift partitions; DMA moves the
                        # odd head's rows to partitions 64..127
                        nc.gpsimd.dma_start(ot[HD:P, h // 2, :], tmp[:])

            # -------- output projection + bias --------
            with (
                tc.tile_pool(name="wp", bufs=3) as wp_pool,
                tc.tile_pool(name="outp", bufs=3) as out_pool,
                tc.tile_pool(name="psE", bufs=4, space="PSUM") as psE,
            ):
                for oi in range(DC):
                    wpt = wp_pool.tile([P, DC, P], f32r)
                    nc.gpsimd.dma_start(wpt[:], wpr[:, oi, :, :])
                    osb = out_pool.tile([P, NT], f32)
                    for nh in range(2):
                        ps = psE.tile([P, 512], f32)
                        for a in range(DC):
                            nc.tensor.matmul(
                                ps[:],
                                wpt[:, a, :],
                                ot[:, a, nh * 512 : (nh + 1) * 512],
                                start=(a == 0),
                                stop=(a == DC - 1),
                            )
                        nc.vector.tensor_scalar_add(
                            osb[:, nh * 512 : (nh + 1) * 512],
                            ps[:],
                            bias_t[:, oi : oi + 1],
                        )
                    nc.gpsimd.dma_start(yT[oi * P : (oi + 1) * P, :], osb[:])

    return nc


def _get_nc():
    if "nc" not in _CACHE:
        _CACHE["nc"] = _build_module()
    return _CACHE["nc"]


def _host_inputs(x, W_qkv, W_proj, b_proj):
    x = np.asarray(x, dtype=np.float32)
    W_qkv = np.asarray(W_qkv, dtype=np.float32)
    W_proj = np.asarray(W_proj, dtype=np.float32)
    b_proj = np.asarray(b_proj, dtype=np.float32)

    wqkvT = W_qkv.T  # [1024, 3072]
    # wqk[p, ct, a, c] = wqkvT[a*128+p, ct*128+c] for q,k channels
    wqk = np.ascontiguousarray(
        wqkvT[:, : 2 * D].reshape(DC, P, 16, P).transpose(1, 2, 0, 3)
    )
    # wv[p, a, cv] = wqkvT[a*128+p, 2048+cv]
    wv = np.ascontiguousarray(wqkvT[:, 2 * D :].reshape(DC, P, D).transpose(1, 0, 2))
    # wpr[p, ot, a, c] = W_proj.T[a*128+p, ot*128+c]
    wpr = np.ascontiguousarray(
        W_proj.T.reshape(DC, P, DC, P).transpose(1, 2, 0, 3)
    )
    biasT = np.ascontiguousarray(b_proj.reshape(DC, P).T)

    in_maps = []
    for i in range(N_CORES):
        in_maps.append(
            {
                "xT": np.ascontiguousarray(x[i].T),
                "wqk": wqk,
                "wv": wv,
                "wpr": wpr,
                "biasT": biasT,
            }
        )
    return in_maps


def _run(in_maps, trace=False):
    from concourse.bass_utils import run_bass_kernel_spmd

    nc = _get_nc()
    return run_bass_kernel_spmd(nc, in_maps, list(range(N_CORES)), trace=trace)


def kernel(x, W_qkv, W_proj, b_proj):
    in_maps = _host_inputs(x, W_qkv, W_proj, b_proj)
    res = _run(in_maps)
    out = np.stack([res.results[i]["yT"].T for i in range(N_CORES)], axis=0)
    return np.ascontiguousarray(out, dtype=np.float32)
